# revision 1
# baseline (speedup 1.0000x reference)
"""AdaptiveConv (GNN message passing) on 8 TRN2 NeuronCores.

Math (the reference simplifies because gamma*2*(1-lambda) == 1):
    deg  = histogram(col) + 1 ; dinv = rsqrt(deg)
    xh   = dinv * x
    spmm(x)[i] = dinv[i] * ( sum_{e: row_e=i} xh[col_e] + xh[i] )
    for 3 iters:  y = spmm(x); d = y - x0; rn = ||d||_row
                  s = relu(rn - lam) / rn;  x = x0 + s*d

Distribution: nodes row-sharded across 8 cores.  Each iteration:
  1. every core computes xh for its shard; FOUR AllGathers (one per
     quarter-of-every-shard sub-table, separate dram tensors) so pass-p
     gathers overlap the remaining collectives (the random graph makes
     the halo dense, so full gathers are optimal)
  2. per-edge gather of 256B source rows (gpsimd.dma_gather, int16 idx
     => 4 sub-tables of <=32767 rows = one "pass" each, <=1024 idx per
     call, single_packet=False).  Edge slots are grouped per dst node
     and padded to a bucket length L; the bucket schedule minimizes
     total 128-slot chunks via suffix-greedy capacity planning with
     node "upgrades" (gathers are DESCRIPTOR-bound at ~8ns/row, so
     slot count is the cost).  Segment-sum via TensorE matmul with
     fixed block-diagonal 0/1 selectors (counts 1-3 get full-width
     M=128/64/42 selectors; counts >=4 get 32-row psum slabs stacked
     4 per tile), fp32 exact
  3. per-pass partial results land in per-pass R tables in HBM
     (psum-schedule order); a fixed-L=1 gather + identity-matmul
     accumulation recombines them into a common node order
  4. proximal step is node-local vector math

Host-side preprocessing only touches edge_index (graph structure): slot
tables, bucket schedule, degree-derived dinv.  All x-dependent compute
runs on device in fp32.
"""

import math
import numpy as np

import concourse.bass as bass
import concourse.mybir as mybir
import concourse.tile as tile
from concourse import bacc
from concourse.bass_utils import run_bass_kernel_spmd

F32 = mybir.dt.float32
I16 = mybir.dt.int16

CORES = 8
D = 64
K_ITERS = 3
LAMBDA_AMP = 0.1
LAM = (1.0 / (2.0 * (1.0 - LAMBDA_AMP))) * LAMBDA_AMP  # = 0.0555...
PAD_ROWS = 12  # zero rows appended to each shard in the gathered table

# (L, M): nodes with count<=L get an aligned L-slot group; M groups per
# 128-slot chunk (M*L <= 128, M <= 32 so psum slabs stack at 32-aligned
# partition offsets).
# first NSPEC buckets are "full-width": one matmul = one psum tile of
# [M, 512] with M > 32 (no slab stacking); R rows 128/chunk uniform
LADDER = [
    (1, 128), (2, 64), (3, 42),
    (4, 32), (5, 25), (6, 21), (7, 18), (8, 16), (9, 14), (10, 12),
    (12, 10), (14, 9), (16, 8), (18, 7), (21, 6), (25, 5), (32, 4),
    (42, 3), (64, 2), (128, 1),
]
NSPEC = 3
NL = len(LADDER)
_LVALS = np.array([l for l, _ in LADDER])

GCALL = 32      # chunks per call (4096 idx; fine with single_packet=False)
GB_CHUNKS = 64  # chunks per gather buffer (matmul consumption block)


def _bucket_of(counts):
    """Vectorized: count (>=1) -> ladder index."""
    return np.searchsorted(_LVALS, counts)


class Plan:
    """Global (core-independent) schedule + per-core data tensors."""

    def __init__(self, N):
        assert N % CORES == 0
        self.N = N
        self.NSH = N // CORES
        assert self.NSH % 4 == 0
        self.QP = self.NSH // 4       # nodes per shard-quarter
        self.SHQ = self.QP + 3        # + zero pad rows per quarter
        self.SUBT = CORES * self.SHQ  # rows per subtable (one quarter of all)
        assert self.SUBT <= 32767, "subtable exceeds int16 index range"
        self.CH = int(math.ceil(self.NSH / 128 / 8)) * 8  # chunks, mult of 8
        self.NT = 128 * self.CH  # padded positions per core


def preprocess(x, edge_index):
    """All graph-structure preprocessing. Returns a Plan."""
    N = x.shape[0]
    P = Plan(N)
    NSH, CH, NT = P.NSH, P.CH, P.NT
    QP, SHQ = P.QP, P.SHQ
    row = np.asarray(edge_index[0], dtype=np.int64)
    col = np.asarray(edge_index[1], dtype=np.int64)

    deg = np.bincount(col, minlength=N).astype(np.float64) + 1.0
    dinv_all = (1.0 / np.sqrt(deg)).astype(np.float32)

    # ---- per-core (node, pass) neighbor lists -------------------------
    core_data = []
    for c in range(CORES):
        m = (row >= c * NSH) & (row < (c + 1) * NSH)
        dl = row[m] - c * NSH
        src = col[m]
        # self loop
        dl = np.concatenate([dl, np.arange(NSH, dtype=np.int64)])
        src = np.concatenate([src, np.arange(c * NSH, (c + 1) * NSH, dtype=np.int64)])
        lcl = src % NSH
        p_of = lcl // QP  # subtable = quarter-of-every-shard
        loc = (src // NSH) * SHQ + (lcl - p_of * QP)  # subtable-local row
        key = dl * 4 + p_of
        order = np.argsort(key, kind="stable")
        key_s = key[order]
        loc_s = loc[order].astype(np.int64)
        cnt = np.bincount(key_s, minlength=NSH * 4).reshape(NSH, 4)
        starts = np.concatenate([[0], np.cumsum(cnt.reshape(-1))])[:-1].reshape(NSH, 4)
        assert cnt.max() <= 128, f"node degree {cnt.max()} exceeds max bucket"
        core_data.append({"cnt": cnt, "starts": starts, "loc_s": loc_s})

    # ---- global schedule -------------------------------------------
    # Minimize total chunks subject to per-core feasibility: a node may
    # be placed in any bucket with L >= its count ("upgrade"), so the
    # binding constraints are the suffix capacities.  Greedy from the
    # largest bucket down is optimal since M is larger at lower buckets.
    nch = np.zeros((4, NL), dtype=np.int64)
    Ms = np.array([m for _, m in LADDER])
    for p in range(4):
        nat = np.zeros((CORES, NL), dtype=np.int64)
        for c in range(CORES):
            cnts = core_data[c]["cnt"][:, p]
            nz = cnts[cnts > 0]
            nat[c] = np.bincount(_bucket_of(nz), minlength=NL)
        suf_need = np.maximum.reduce(
            [np.cumsum(nat[c][::-1])[::-1] for c in range(CORES)])
        cap = 0
        for t in range(NL - 1, -1, -1):
            deficit = max(0, int(suf_need[t]) - cap)
            nch[p][t] = -(-deficit // int(Ms[t]))
            cap += int(Ms[t] * nch[p][t])
    # pad special section to mult of 8 chunks, standard to mult of 32
    n_spec = np.zeros(4, dtype=np.int64)
    for p in range(4):
        cs = int(nch[p][:NSPEC].sum())
        pad = (-cs) % 8
        nch[p][NSPEC - 1] += pad
        n_spec[p] = cs + pad
        cstd = int(nch[p][NSPEC:].sum())
        pad = (-cstd) % 32
        nch[p][NSPEC] += pad
    ch_tot = nch.sum(axis=1)
    P.nch = nch
    P.ch_tot = ch_tot
    P.n_spec = n_spec
    # R rows: 1024 per staged psum tile (special: 8 chunks x 128 rows;
    # standard: 32 chunks x 32 rows)
    P.rrows = [int(1024 * (n_spec[p] // 8 + (ch_tot[p] - n_spec[p]) // 32))
               for p in range(4)]
    for p in range(4):
        assert P.rrows[p] + 1 <= 32767, f"R table {p} too big: {P.rrows[p]}"

    # ---- per-core slot arrays + recombine tables ----------------------
    per_core = []
    for c in range(CORES):
        cd = core_data[c]
        cnt, starts, loc_s = cd["cnt"], cd["starts"], cd["loc_s"]
        slots_main = []
        kp = np.full((4, NSH), -1, dtype=np.int64)  # R_p row of node l
        for p in range(4):
            slots_p = np.full(int(ch_tot[p]) * 128, QP, dtype=np.int16)  # ZROW=QP
            cnts = cnt[:, p]
            nodes_nz = np.nonzero(cnts > 0)[0]
            # capacity-fill: largest buckets take the largest counts;
            # smaller nodes may be upgraded into leftover capacity
            order = nodes_nz[np.argsort(-cnts[nodes_nz], kind="stable")]
            chunk_base_of = np.concatenate([[0], np.cumsum(nch[p])])
            ptr = 0
            for bi in range(NL - 1, -1, -1):
                L, M = LADDER[bi]
                cap_b = int(nch[p][bi]) * M
                take = order[ptr:ptr + cap_b]
                ptr += cap_b
                if len(take) == 0:
                    continue
                assert cnts[take[0]] <= L, "schedule infeasible"
                cb = int(chunk_base_of[bi])
                nsp = int(P.n_spec[p])
                for j, l in enumerate(take):
                    chk = cb + j // M
                    g = j % M
                    n = int(cnts[l])
                    s0 = chk * 128 + g * L
                    slots_p[s0:s0 + n] = loc_s[starts[l, p]:starts[l, p] + n]
                    if bi < NSPEC:
                        # full-width: tile = chk//8, row = g*8 + cc
                        kp[p, l] = (chk // 8) * 1024 + g * 8 + (chk % 8)
                    else:
                        # 4-slab: relative to the standard section
                        cs = chk - nsp
                        mm = cs // 8
                        kp[p, l] = (nsp // 8) * 1024 + (mm // 4) * 1024                             + ((mm % 4) * 32 + g) * 8 + (cs % 8)
            assert ptr >= len(order), "nodes left unplaced"
            slots_main.append(slots_p)
        slots_main = np.concatenate(slots_main)

        # recombine: slot (ch, e) -> position r = e*CH + ch -> node l=r
        e_idx = np.arange(NT, dtype=np.int64)
        ch_i = e_idx // 128
        e_i = e_idx % 128
        pos = e_i * CH + ch_i
        slots_rec = []
        for p in range(4):
            zr = P.rrows[p]
            v = np.full(NT, zr, dtype=np.int64)
            real = pos < NSH
            l_of = pos[real]
            kv = kp[p, l_of]
            v[real] = np.where(kv >= 0, kv, zr)
            slots_rec.append(v.astype(np.int16))
        slots_rec = np.concatenate(slots_rec)

        # x0 / dinv tiles in position layout [128, CH*D] / [128, CH]
        xt = np.zeros((128 * CH, D), dtype=np.float32)
        xt[:NSH] = x[c * NSH:(c + 1) * NSH]
        dt_ = np.zeros(128 * CH, dtype=np.float32)
        dt_[:NSH] = dinv_all[c * NSH:(c + 1) * NSH]
        per_core.append({
            "x0": np.ascontiguousarray(xt.reshape(128, CH * D)),
            "dinv": np.ascontiguousarray(dt_.reshape(128, CH)),
            "slots_main": _wrap16(slots_main),
            "slots_rec": _wrap16(slots_rec),
        })
    P.per_core = per_core

    # selector blob: full-width buckets get M columns, standard get 32
    widths = [128 if bi < NSPEC else 32 for bi, (L, M) in enumerate(LADDER)]
    soff = np.concatenate([[0], np.cumsum(widths)]).astype(int)
    sel = np.zeros((128, int(soff[-1]) + 128), dtype=np.float32)
    e = np.arange(128)
    for bi, (L, M) in enumerate(LADDER):
        g = e // L
        ok = g < M
        sel[e[ok], soff[bi] + g[ok]] = 1.0
    sel[e, int(soff[-1]) + e] = 1.0  # identity for recombine
    P.sel = sel
    P.soff = soff
    return P


def _wrap16(a):
    """int16 1-D array -> [128, ceil(n/16)] wrapped layout: value at
    (p, s) = a[s*16 + p%16], replicated across the 8 Q7 core stripes."""
    n = len(a)
    n16 = int(math.ceil(n / 16)) * 16
    b = np.zeros(n16, dtype=np.int16)
    b[:n] = a
    w = b.reshape(-1, 16).T
    return np.ascontiguousarray(np.tile(w, (8, 1)))


# ======================================================================
# Bass kernel builder
# ======================================================================

def build_kernel(P: Plan):
    NSH, SUBT, CH, NT = P.NSH, P.SUBT, P.CH, P.NT
    QP, SHQ = P.QP, P.SHQ
    CHD = CH * D
    TOTM = P.per_core[0]["slots_main"].shape[1]
    TOTR = P.per_core[0]["slots_rec"].shape[1]
    n_rec_tiles = CH // 8

    nc = bacc.Bacc(None, target_bir_lowering=False)

    x0_p = nc.declare_dram_parameter("x0", [128, CHD], F32, isOutput=False)
    dinv_p = nc.declare_dram_parameter("dinv", [128, CH], F32, isOutput=False)
    SELW = P.sel.shape[1]
    sel_p = nc.declare_dram_parameter("selectors", [128, SELW], F32, isOutput=False)
    sm_p = nc.declare_dram_parameter("slots_main", [128, TOTM], I16, isOutput=False)
    sr_p = nc.declare_dram_parameter("slots_rec", [128, TOTR], I16, isOutput=False)
    out_p = nc.declare_dram_parameter("out", [128, CHD], F32, isOutput=True)

    bounce_q = [nc.dram_tensor(f"bounce{p}", [SHQ, D], F32) for p in range(4)]
    xh_q = [nc.dram_tensor(f"xhq{p}", [SUBT, D], F32, addr_space="Shared")
            for p in range(4)]
    rp = [nc.dram_tensor(f"rp{p}", [P.rrows[p] + 1, D], F32) for p in range(4)]

    with tile.TileContext(nc) as tc:
        with (
            tc.tile_pool(name="persist", bufs=1) as pp,
            tc.tile_pool(name="gmain", bufs=2) as gp,
            tc.tile_pool(name="grec", bufs=8) as grp,
            tc.tile_pool(name="idx", bufs=2) as ip,
            tc.tile_pool(name="stage", bufs=2) as sp,
            tc.tile_pool(name="psum", bufs=2, space="PSUM") as psp,
            tc.tile_pool(name="psumr", bufs=2, space="PSUM") as psrp,
        ):
            B0 = pp.tile([128, CHD], F32)
            B1 = pp.tile([128, CHD], F32)
            B2 = pp.tile([128, CHD], F32)
            DINV = pp.tile([128, CH], F32)
            SEL = pp.tile([128, SELW], F32)
            RN = pp.tile([128, CH], F32)
            SC = pp.tile([128, CH], F32)
            RC = pp.tile([128, CH], F32)
            ZT = pp.tile([1, max(PAD_ROWS * D, D)], F32)

            nc.sync.dma_start(out=B0[:], in_=x0_p[:])
            nc.sync.dma_start(out=DINV[:], in_=dinv_p[:])
            nc.sync.dma_start(out=SEL[:], in_=sel_p[:])
            nc.vector.memset(ZT[:], 0.0)
            for p in range(4):
                nc.sync.dma_start(
                    out=bounce_q[p][QP:SHQ, :].rearrange("(o r) f -> o (r f)", o=1),
                    in_=ZT[:1, :3 * D])
                nc.sync.dma_start(
                    out=rp[p][P.rrows[p]:P.rrows[p] + 1, :], in_=ZT[:1, :D])

            def bcast(t, cols):
                """[128, cols] tile -> [128, cols, D] zero-stride AP."""
                return t[:].rearrange("p (c o) -> p c o", o=1).to_broadcast([128, cols, D])

            def bounce_pieces(p):
                """affine DMA pieces covering positions [p*QP, (p+1)*QP)."""
                pieces = []
                a, b = p * QP, (p + 1) * QP
                base = 0
                # partial head
                if a % CH:
                    g = a // CH
                    take = min(CH - a % CH, b - a)
                    pieces.append((base, g, g + 1, a % CH, a % CH + take))
                    base += take
                    a += take
                # full middle
                gm0, gm1 = a // CH, b // CH
                if gm1 > gm0:
                    pieces.append((base, gm0, gm1, 0, CH))
                    base += (gm1 - gm0) * CH
                    a = gm1 * CH
                if a < b:  # partial tail
                    pieces.append((base, b // CH, b // CH + 1, 0, b - a))
                return pieces

            cur = B0
            for it in range(K_ITERS):
                # ---- xh = dinv * x_cur -> B1 -> bounce -> AllGather ----
                nc.vector.tensor_tensor(
                    out=B1[:].rearrange("p (c f) -> p c f", f=D),
                    in0=cur[:].rearrange("p (c f) -> p c f", f=D),
                    in1=bcast(DINV, CH),
                    op=mybir.AluOpType.mult,
                )
                for p in range(4):
                    for (rbase, g0, g1, c0, c1) in bounce_pieces(p):
                        n = (g1 - g0) * (c1 - c0)
                        nc.sync.dma_start(
                            out=bounce_q[p][rbase:rbase + n, :]
                            .rearrange("(g c) f -> g c f", g=g1 - g0),
                            in_=B1[g0:g1, c0 * D:c1 * D]
                            .rearrange("g (c f) -> g c f", f=D),
                        )
                    nc.gpsimd.collective_compute(
                        "AllGather",
                        mybir.AluOpType.bypass,
                        replica_groups=[list(range(CORES))],
                        ins=[bounce_q[p][:, :]],
                        outs=[xh_q[p][:, :]],
                    )

                # ---- main passes: gather + selector matmuls -> R_p ----
                for p in range(4):
                    pass_chunk0 = int(np.sum(P.ch_tot[:p]))
                    chunks_p = int(P.ch_tot[p])
                    chunk_bucket = np.repeat(np.arange(NL), P.nch[p])
                    assert len(chunk_bucket) == chunks_p
                    n_blocks = int(math.ceil(chunks_p / GB_CHUNKS))
                    n_spec_p = int(P.n_spec[p])
                    mm_in_tile = 0
                    ps_t = None
                    stage_t = None
                    stage_tiles = 0
                    stage_row0 = 0
                    tiles_done = 0
                    for blk in range(n_blocks):
                        cb0 = blk * GB_CHUNKS
                        nch_b = min(GB_CHUNKS, chunks_p - cb0)
                        it_t = ip.tile([128, GB_CHUNKS * 8], I16, tag="idxm")
                        s0 = (pass_chunk0 + cb0) * 8
                        nc.sync.dma_start(out=it_t[:, :nch_b * 8],
                                          in_=sm_p[:, s0:s0 + nch_b * 8])
                        g_t = gp.tile([128, GB_CHUNKS, D], F32, tag="gmain")
                        # split the block into <=GCALL-chunk gather calls
                        for ca in range(0, nch_b, GCALL):
                            cb = min(ca + GCALL, nch_b)
                            nidx = (cb - ca) * 128
                            nc.gpsimd.dma_gather(
                                g_t[:, ca:cb, :],
                                xh_q[p][:, :],
                                it_t[:, ca * 8:cb * 8],
                                nidx,
                                nidx,
                                D,
                                elem_step=D,
                                single_packet=False,
                            )
                        for mi in range(nch_b // 8):
                            chk = cb0 + mi * 8
                            bi = int(chunk_bucket[chk])
                            if (chunk_bucket[chk:chk + 8] == bi).all():
                                spans = [(0, 8, bi)]
                            else:
                                spans = []
                                j0 = 0
                                for j in range(1, 8):
                                    if chunk_bucket[chk + j] != chunk_bucket[chk + j0]:
                                        spans.append((j0, j, int(chunk_bucket[chk + j0])))
                                        j0 = j
                                spans.append((j0, 8, int(chunk_bucket[chk + j0])))
                            if chk < n_spec_p:
                                # full-width: one mm group = one psum tile
                                ps_t = psp.tile([128, 512], F32, tag="psm")
                                for (ja, jb, bspan) in spans:
                                    nc.tensor.matmul(
                                        out=ps_t[:, ja * 64:jb * 64],
                                        lhsT=SEL[:, int(P.soff[bspan]):int(P.soff[bspan]) + 128],
                                        rhs=g_t[:, mi * 8 + ja:mi * 8 + jb, :],
                                        start=True, stop=True,
                                    )
                                tile_complete = True
                            else:
                                if mm_in_tile == 0:
                                    ps_t = psp.tile([128, 512], F32, tag="psm")
                                slab = mm_in_tile
                                for (ja, jb, bspan) in spans:
                                    nc.tensor.matmul(
                                        out=ps_t[32 * slab:32 * slab + 32, ja * 64:jb * 64],
                                        lhsT=SEL[:, int(P.soff[bspan]):int(P.soff[bspan]) + 32],
                                        rhs=g_t[:, mi * 8 + ja:mi * 8 + jb, :],
                                        start=True, stop=True,
                                        tile_position=(0, 32 * slab),
                                    )
                                mm_in_tile += 1
                                tile_complete = mm_in_tile == 4
                                if tile_complete:
                                    mm_in_tile = 0
                            if tile_complete:
                                if stage_tiles == 0:
                                    stage_t = sp.tile([128, 4 * 512], F32, tag="stg")
                                    stage_row0 = tiles_done * 1024
                                nc.vector.tensor_copy(
                                    out=stage_t[:, stage_tiles * 512:(stage_tiles + 1) * 512],
                                    in_=ps_t[:],
                                )
                                stage_tiles += 1
                                tiles_done += 1
                                flush = (stage_tiles == 4) or (chk + 8 == chunks_p)
                                if flush:
                                    # row(k, q, cc) = row0 + k*1024 + q*8 + cc
                                    nc.sync.dma_start(
                                        out=rp[p][stage_row0:stage_row0 + stage_tiles * 1024, :]
                                        .rearrange("(k q cc) f -> q k cc f", q=128, cc=8),
                                        in_=stage_t[:, :stage_tiles * 512]
                                        .rearrange("q (k cc f) -> q k cc f", cc=8, f=D),
                                    )
                                    stage_tiles = 0
                    assert mm_in_tile == 0, "pass chunks not multiple of 32"

                # ---- recombine: 4 gathers + identity matmul per tile ----
                for t in range(n_rec_tiles):
                    ps2 = psrp.tile([128, 512], F32, tag="psr")
                    for p in range(4):
                        it2 = ip.tile([128, 64], I16, tag="idxr")
                        s0 = (p * NT + t * 1024) // 16
                        nc.sync.dma_start(out=it2[:], in_=sr_p[:, s0:s0 + 64])
                        g2 = grp.tile([128, 8, D], F32, tag="grec")
                        nc.gpsimd.dma_gather(
                            g2[:], rp[p][:, :], it2[:], 1024, 1024, D,
                            elem_step=D, single_packet=False,
                        )
                        nc.tensor.matmul(
                            out=ps2[:],
                            lhsT=SEL[:, SELW - 128:SELW],
                            rhs=g2[:],
                            start=(p == 0), stop=(p == 3),
                        )
                    nc.vector.tensor_copy(out=B2[:, t * 512:(t + 1) * 512], in_=ps2[:])

                # ---- proximal (node-local) ----
                b0_3 = B0[:].rearrange("p (c f) -> p c f", f=D)
                b1_3 = B1[:].rearrange("p (c f) -> p c f", f=D)
                b2_3 = B2[:].rearrange("p (c f) -> p c f", f=D)
                dv3 = bcast(DINV, CH)
                nc.vector.tensor_tensor(out=b2_3, in0=b2_3, in1=dv3, op=mybir.AluOpType.mult)
                nc.vector.tensor_tensor(out=b1_3, in0=b2_3, in1=b0_3, op=mybir.AluOpType.subtract)
                nc.vector.tensor_tensor(out=b2_3, in0=b1_3, in1=b1_3, op=mybir.AluOpType.mult)
                nc.vector.tensor_reduce(
                    out=RN[:], in_=b2_3, axis=mybir.AxisListType.X, op=mybir.AluOpType.add,
                )
                nc.scalar.sqrt(RN[:], RN[:])
                nc.vector.tensor_scalar_add(RC[:], RN[:], 1e-30)
                nc.vector.reciprocal(RC[:], RC[:])
                nc.vector.tensor_scalar_add(SC[:], RN[:], -LAM)
                nc.vector.tensor_scalar_max(SC[:], SC[:], 0.0)
                nc.vector.tensor_tensor(out=SC[:], in0=SC[:], in1=RC[:], op=mybir.AluOpType.mult)
                nc.vector.tensor_tensor(out=b1_3, in0=b1_3, in1=bcast(SC, CH), op=mybir.AluOpType.mult)
                nc.vector.tensor_tensor(out=b2_3, in0=b1_3, in1=b0_3, op=mybir.AluOpType.add)
                cur = B2

            nc.sync.dma_start(out=out_p[:], in_=B2[:])

    return nc


# ======================================================================
# entry point
# ======================================================================

def _build_and_run(x, edge_index, trace=False):
    x = np.ascontiguousarray(np.asarray(x, dtype=np.float32))
    P = preprocess(x, edge_index)
    nc = build_kernel(P)
    nc.finalize()  # Bacc defers register allocation to compile()
    in_maps = []
    for c in range(CORES):
        d = P.per_core[c]
        in_maps.append({
            "x0": d["x0"], "dinv": d["dinv"], "selectors": P.sel,
            "slots_main": d["slots_main"], "slots_rec": d["slots_rec"],
        })
    res = run_bass_kernel_spmd(nc, in_maps, list(range(CORES)), trace=trace)
    outs = []
    for c in range(CORES):
        o = res.results[c]["out"].reshape(128 * P.CH, D)[:P.NSH]
        outs.append(o)
    return np.concatenate(outs, axis=0), res


def kernel(x, edge_index):
    out, _ = _build_and_run(x, edge_index, trace=False)
    return out



# revision 4
# speedup vs baseline: 1.8677x; 1.8677x over previous
"""AdaptiveConv (GNN message passing) on 8 TRN2 NeuronCores.

Math (the reference simplifies because gamma*2*(1-lambda) == 1):
    deg  = histogram(col) + 1 ; dinv = rsqrt(deg)
    xh   = dinv * x
    spmm(x)[i] = dinv[i] * ( sum_{e: row_e=i} xh[col_e] + xh[i] )
    for 3 iters:  y = spmm(x); d = y - x0; rn = ||d||_row
                  s = relu(rn - lam) / rn;  x = x0 + s*d

Distribution: nodes row-sharded across 8 cores.  Each iteration:
  1. every core computes xh for its shard; FOUR AllGathers (one per
     quarter-of-every-shard sub-table, separate dram tensors) so pass-p
     gathers overlap the remaining collectives (the random graph makes
     the halo dense, so full gathers are optimal)
  2. per-edge gather of 256B source rows (gpsimd.dma_gather, int16 idx
     => 4 sub-tables of <=32767 rows = one "pass" each, <=1024 idx per
     call, single_packet=False).  Edge slots are grouped per dst node
     and padded to a bucket length L; the bucket schedule minimizes
     total 128-slot chunks via suffix-greedy capacity planning with
     node "upgrades" (gathers are DESCRIPTOR-bound at ~8ns/row, so
     slot count is the cost).  Segment-sum via TensorE matmul with
     fixed block-diagonal 0/1 selectors (counts 1-3 get full-width
     M=128/64/42 selectors; counts >=4 get 32-row psum slabs stacked
     4 per tile), fp32 exact
  3. per-pass partial results land in per-pass R tables in HBM
     (psum-schedule order); a fixed-L=1 gather + identity-matmul
     accumulation recombines them into a common node order
  4. proximal step is node-local vector math

Host-side preprocessing only touches edge_index (graph structure): slot
tables, bucket schedule, degree-derived dinv.  All x-dependent compute
runs on device in fp32.
"""

import math
import numpy as np

import concourse.bass as bass
import concourse.mybir as mybir
import concourse.tile as tile
from concourse import bacc
from concourse.bass_utils import run_bass_kernel_spmd

F32 = mybir.dt.float32
I16 = mybir.dt.int16

CORES = 8
D = 64
K_ITERS = 3
LAMBDA_AMP = 0.1
LAM = (1.0 / (2.0 * (1.0 - LAMBDA_AMP))) * LAMBDA_AMP  # = 0.0555...
PAD_ROWS = 12  # zero rows appended to each shard in the gathered table

# (L, M): nodes with count<=L get an aligned L-slot group; M groups per
# 128-slot chunk (M*L <= 128, M <= 32 so psum slabs stack at 32-aligned
# partition offsets).
# first NSPEC buckets are "full-width": one matmul = one psum tile of
# [M, 512] with M > 32 (no slab stacking); R rows 128/chunk uniform
LADDER = [
    (1, 128), (2, 64), (3, 42),
    (4, 32), (5, 25), (6, 21), (7, 18), (8, 16), (9, 14), (10, 12),
    (12, 10), (14, 9), (16, 8), (18, 7), (21, 6), (25, 5), (32, 4),
    (42, 3), (64, 2), (128, 1),
]
NSPEC = 3
NL = len(LADDER)
_LVALS = np.array([l for l, _ in LADDER])

GCALL = 32      # chunks per call (4096 idx; fine with single_packet=False)
GB_CHUNKS = 64  # chunks per gather buffer (matmul consumption block)


def _bucket_of(counts):
    """Vectorized: count (>=1) -> ladder index."""
    return np.searchsorted(_LVALS, counts)


class Plan:
    """Global (core-independent) schedule + per-core data tensors."""

    def __init__(self, N):
        assert N % CORES == 0
        self.N = N
        self.NSH = N // CORES
        assert self.NSH % 4 == 0
        self.QP = self.NSH // 4       # nodes per shard-quarter
        self.SHQ = self.QP + 3        # + zero pad rows per quarter
        self.SUBT = CORES * self.SHQ  # rows per subtable (one quarter of all)
        assert self.SUBT <= 32767, "subtable exceeds int16 index range"
        self.CH = int(math.ceil(self.NSH / 128 / 8)) * 8  # chunks, mult of 8
        self.NT = 128 * self.CH  # padded positions per core


def preprocess(x, edge_index):
    """All graph-structure preprocessing. Returns a Plan."""
    N = x.shape[0]
    P = Plan(N)
    NSH, CH, NT = P.NSH, P.CH, P.NT
    QP, SHQ = P.QP, P.SHQ
    row = np.asarray(edge_index[0], dtype=np.int64)
    col = np.asarray(edge_index[1], dtype=np.int64)

    deg = np.bincount(col, minlength=N).astype(np.float64) + 1.0
    dinv_all = (1.0 / np.sqrt(deg)).astype(np.float32)

    # ---- per-core (node, pass) neighbor lists -------------------------
    core_data = []
    for c in range(CORES):
        m = (row >= c * NSH) & (row < (c + 1) * NSH)
        dl = row[m] - c * NSH
        src = col[m]
        # self loop
        dl = np.concatenate([dl, np.arange(NSH, dtype=np.int64)])
        src = np.concatenate([src, np.arange(c * NSH, (c + 1) * NSH, dtype=np.int64)])
        lcl = src % NSH
        p_of = lcl // QP  # subtable = quarter-of-every-shard
        loc = (src // NSH) * SHQ + (lcl - p_of * QP)  # subtable-local row
        key = dl * 4 + p_of
        order = np.argsort(key, kind="stable")
        key_s = key[order]
        loc_s = loc[order].astype(np.int64)
        cnt = np.bincount(key_s, minlength=NSH * 4).reshape(NSH, 4)
        starts = np.concatenate([[0], np.cumsum(cnt.reshape(-1))])[:-1].reshape(NSH, 4)
        assert cnt.max() <= 128, f"node degree {cnt.max()} exceeds max bucket"
        core_data.append({"cnt": cnt, "starts": starts, "loc_s": loc_s})

    # ---- global schedule -------------------------------------------
    # Minimize total chunks subject to per-core feasibility: a node may
    # be placed in any bucket with L >= its count ("upgrade"), so the
    # binding constraints are the suffix capacities.  Greedy from the
    # largest bucket down is optimal since M is larger at lower buckets.
    nch = np.zeros((4, NL), dtype=np.int64)
    Ms = np.array([m for _, m in LADDER])
    for p in range(4):
        nat = np.zeros((CORES, NL), dtype=np.int64)
        for c in range(CORES):
            cnts = core_data[c]["cnt"][:, p]
            nz = cnts[cnts > 0]
            nat[c] = np.bincount(_bucket_of(nz), minlength=NL)
        suf_need = np.maximum.reduce(
            [np.cumsum(nat[c][::-1])[::-1] for c in range(CORES)])
        cap = 0
        for t in range(NL - 1, -1, -1):
            deficit = max(0, int(suf_need[t]) - cap)
            nch[p][t] = -(-deficit // int(Ms[t]))
            cap += int(Ms[t] * nch[p][t])
    # pad special section to mult of 8 chunks, standard to mult of 32
    n_spec = np.zeros(4, dtype=np.int64)
    for p in range(4):
        cs = int(nch[p][:NSPEC].sum())
        pad = (-cs) % 8
        nch[p][NSPEC - 1] += pad
        n_spec[p] = cs + pad
        cstd = int(nch[p][NSPEC:].sum())
        pad = (-cstd) % 32
        nch[p][NSPEC] += pad
    ch_tot = nch.sum(axis=1)
    P.nch = nch
    P.ch_tot = ch_tot
    P.n_spec = n_spec
    # R rows: 1024 per staged psum tile (special: 8 chunks x 128 rows;
    # standard: 32 chunks x 32 rows)
    P.rrows = [int(1024 * (n_spec[p] // 8 + (ch_tot[p] - n_spec[p]) // 32))
               for p in range(4)]
    for p in range(4):
        assert P.rrows[p] + 1 <= 32767, f"R table {p} too big: {P.rrows[p]}"

    # ---- per-core slot arrays + recombine tables ----------------------
    per_core = []
    for c in range(CORES):
        cd = core_data[c]
        cnt, starts, loc_s = cd["cnt"], cd["starts"], cd["loc_s"]
        slots_main = []
        kp = np.full((4, NSH), -1, dtype=np.int64)  # R_p row of node l
        for p in range(4):
            slots_p = np.full(int(ch_tot[p]) * 128, QP, dtype=np.int16)  # ZROW=QP
            cnts = cnt[:, p]
            nodes_nz = np.nonzero(cnts > 0)[0]
            # capacity-fill: largest buckets take the largest counts;
            # smaller nodes may be upgraded into leftover capacity
            order = nodes_nz[np.argsort(-cnts[nodes_nz], kind="stable")]
            chunk_base_of = np.concatenate([[0], np.cumsum(nch[p])])
            ptr = 0
            for bi in range(NL - 1, -1, -1):
                L, M = LADDER[bi]
                cap_b = int(nch[p][bi]) * M
                take = order[ptr:ptr + cap_b]
                ptr += cap_b
                if len(take) == 0:
                    continue
                assert cnts[take[0]] <= L, "schedule infeasible"
                cb = int(chunk_base_of[bi])
                nsp = int(P.n_spec[p])
                for j, l in enumerate(take):
                    chk = cb + j // M
                    g = j % M
                    n = int(cnts[l])
                    s0 = chk * 128 + g * L
                    slots_p[s0:s0 + n] = loc_s[starts[l, p]:starts[l, p] + n]
                    if bi < NSPEC:
                        # full-width: tile = chk//8, row = g*8 + cc
                        kp[p, l] = (chk // 8) * 1024 + g * 8 + (chk % 8)
                    else:
                        # 4-slab: relative to the standard section
                        cs = chk - nsp
                        mm = cs // 8
                        kp[p, l] = (nsp // 8) * 1024 + (mm // 4) * 1024                             + ((mm % 4) * 32 + g) * 8 + (cs % 8)
            assert ptr >= len(order), "nodes left unplaced"
            slots_main.append(slots_p)
        slots_main = np.concatenate(slots_main)

        # recombine: slot (ch, e) -> position r = e*CH + ch -> node l=r
        e_idx = np.arange(NT, dtype=np.int64)
        ch_i = e_idx // 128
        e_i = e_idx % 128
        pos = e_i * CH + ch_i
        slots_rec = []
        for p in range(4):
            zr = P.rrows[p]
            v = np.full(NT, zr, dtype=np.int64)
            real = pos < NSH
            l_of = pos[real]
            kv = kp[p, l_of]
            v[real] = np.where(kv >= 0, kv, zr)
            slots_rec.append(v.astype(np.int16))
        slots_rec = np.concatenate(slots_rec)

        # x0 / dinv tiles in position layout [128, CH*D] / [128, CH]
        xt = np.zeros((128 * CH, D), dtype=np.float32)
        xt[:NSH] = x[c * NSH:(c + 1) * NSH]
        dt_ = np.zeros(128 * CH, dtype=np.float32)
        dt_[:NSH] = dinv_all[c * NSH:(c + 1) * NSH]
        per_core.append({
            "x0": np.ascontiguousarray(xt.reshape(128, CH * D)),
            "dinv": np.ascontiguousarray(dt_.reshape(128, CH)),
            "slots_main": _wrap16(slots_main),
            "slots_rec": _wrap16(slots_rec),
        })
    P.per_core = per_core

    # selector blob: full-width buckets get M columns, standard get 32
    widths = [128 if bi < NSPEC else 32 for bi, (L, M) in enumerate(LADDER)]
    soff = np.concatenate([[0], np.cumsum(widths)]).astype(int)
    sel = np.zeros((128, int(soff[-1]) + 128), dtype=np.float32)
    e = np.arange(128)
    for bi, (L, M) in enumerate(LADDER):
        g = e // L
        ok = g < M
        sel[e[ok], soff[bi] + g[ok]] = 1.0
    sel[e, int(soff[-1]) + e] = 1.0  # identity for recombine
    P.sel = sel
    P.soff = soff
    return P


def _wrap16(a):
    """int16 1-D array -> [128, ceil(n/16)] wrapped layout: value at
    (p, s) = a[s*16 + p%16], replicated across the 8 Q7 core stripes."""
    n = len(a)
    n16 = int(math.ceil(n / 16)) * 16
    b = np.zeros(n16, dtype=np.int16)
    b[:n] = a
    w = b.reshape(-1, 16).T
    return np.ascontiguousarray(np.tile(w, (8, 1)))


# ======================================================================
# Bass kernel builder
# ======================================================================

def build_kernel(P: Plan):
    NSH, SUBT, CH, NT = P.NSH, P.SUBT, P.CH, P.NT
    QP, SHQ = P.QP, P.SHQ
    CHD = CH * D
    TOTM = P.per_core[0]["slots_main"].shape[1]
    TOTR = P.per_core[0]["slots_rec"].shape[1]
    n_rec_tiles = CH // 8

    nc = bacc.Bacc(None, target_bir_lowering=False, num_swdge_queues=4)
    qrr = [0]  # round-robin SWDGE queue counter

    def next_q():
        q = qrr[0] & 3
        qrr[0] += 1
        return q

    x0_p = nc.declare_dram_parameter("x0", [128, CHD], F32, isOutput=False)
    dinv_p = nc.declare_dram_parameter("dinv", [128, CH], F32, isOutput=False)
    SELW = P.sel.shape[1]
    sel_p = nc.declare_dram_parameter("selectors", [128, SELW], F32, isOutput=False)
    sm_p = nc.declare_dram_parameter("slots_main", [128, TOTM], I16, isOutput=False)
    sr_p = nc.declare_dram_parameter("slots_rec", [128, TOTR], I16, isOutput=False)
    out_p = nc.declare_dram_parameter("out", [128, CHD], F32, isOutput=True)

    bounce_q = [nc.dram_tensor(f"bounce{p}", [SHQ, D], F32) for p in range(4)]
    xh_q = [nc.dram_tensor(f"xhq{p}", [SUBT, D], F32, addr_space="Shared")
            for p in range(4)]
    rp = [nc.dram_tensor(f"rp{p}", [P.rrows[p] + 1, D], F32) for p in range(4)]

    with tile.TileContext(nc) as tc:
        with (
            tc.tile_pool(name="persist", bufs=1) as pp,
            tc.tile_pool(name="gmain", bufs=2) as gp,
            tc.tile_pool(name="grec", bufs=8) as grp,
            tc.tile_pool(name="idx", bufs=2) as ip,
            tc.tile_pool(name="stage", bufs=2) as sp,
            tc.tile_pool(name="psum", bufs=2, space="PSUM") as psp,
            tc.tile_pool(name="psumr", bufs=2, space="PSUM") as psrp,
        ):
            B0 = pp.tile([128, CHD], F32)
            B1 = pp.tile([128, CHD], F32)
            B2 = pp.tile([128, CHD], F32)
            DINV = pp.tile([128, CH], F32)
            SEL = pp.tile([128, SELW], F32)
            RN = pp.tile([128, CH], F32)
            SC = pp.tile([128, CH], F32)
            RC = pp.tile([128, CH], F32)
            ZT = pp.tile([1, max(PAD_ROWS * D, D)], F32)

            nc.sync.dma_start(out=B0[:], in_=x0_p[:])
            nc.sync.dma_start(out=DINV[:], in_=dinv_p[:])
            nc.sync.dma_start(out=SEL[:], in_=sel_p[:])
            nc.vector.memset(ZT[:], 0.0)
            for p in range(4):
                nc.sync.dma_start(
                    out=bounce_q[p][QP:SHQ, :].rearrange("(o r) f -> o (r f)", o=1),
                    in_=ZT[:1, :3 * D])
                nc.sync.dma_start(
                    out=rp[p][P.rrows[p]:P.rrows[p] + 1, :], in_=ZT[:1, :D])

            def bcast(t, cols):
                """[128, cols] tile -> [128, cols, D] zero-stride AP."""
                return t[:].rearrange("p (c o) -> p c o", o=1).to_broadcast([128, cols, D])

            def bounce_pieces(p):
                """affine DMA pieces covering positions [p*QP, (p+1)*QP)."""
                pieces = []
                a, b = p * QP, (p + 1) * QP
                base = 0
                # partial head
                if a % CH:
                    g = a // CH
                    take = min(CH - a % CH, b - a)
                    pieces.append((base, g, g + 1, a % CH, a % CH + take))
                    base += take
                    a += take
                # full middle
                gm0, gm1 = a // CH, b // CH
                if gm1 > gm0:
                    pieces.append((base, gm0, gm1, 0, CH))
                    base += (gm1 - gm0) * CH
                    a = gm1 * CH
                if a < b:  # partial tail
                    pieces.append((base, b // CH, b // CH + 1, 0, b - a))
                return pieces

            cur = B0
            for it in range(K_ITERS):
                # ---- xh = dinv * x_cur -> B1 -> bounce -> AllGather ----
                nc.vector.tensor_tensor(
                    out=B1[:].rearrange("p (c f) -> p c f", f=D),
                    in0=cur[:].rearrange("p (c f) -> p c f", f=D),
                    in1=bcast(DINV, CH),
                    op=mybir.AluOpType.mult,
                )
                for p in range(4):
                    for (rbase, g0, g1, c0, c1) in bounce_pieces(p):
                        n = (g1 - g0) * (c1 - c0)
                        nc.sync.dma_start(
                            out=bounce_q[p][rbase:rbase + n, :]
                            .rearrange("(g c) f -> g c f", g=g1 - g0),
                            in_=B1[g0:g1, c0 * D:c1 * D]
                            .rearrange("g (c f) -> g c f", f=D),
                        )
                    nc.gpsimd.collective_compute(
                        "AllGather",
                        mybir.AluOpType.bypass,
                        replica_groups=[list(range(CORES))],
                        ins=[bounce_q[p][:, :]],
                        outs=[xh_q[p][:, :]],
                    )

                # ---- main passes: gather + selector matmuls -> R_p ----
                for p in range(4):
                    pass_chunk0 = int(np.sum(P.ch_tot[:p]))
                    chunks_p = int(P.ch_tot[p])
                    chunk_bucket = np.repeat(np.arange(NL), P.nch[p])
                    assert len(chunk_bucket) == chunks_p
                    n_blocks = int(math.ceil(chunks_p / GB_CHUNKS))
                    n_spec_p = int(P.n_spec[p])
                    mm_in_tile = 0
                    ps_t = None
                    stage_t = None
                    stage_tiles = 0
                    stage_row0 = 0
                    tiles_done = 0
                    for blk in range(n_blocks):
                        cb0 = blk * GB_CHUNKS
                        nch_b = min(GB_CHUNKS, chunks_p - cb0)
                        it_t = ip.tile([128, GB_CHUNKS * 8], I16, tag="idxm")
                        s0 = (pass_chunk0 + cb0) * 8
                        nc.sync.dma_start(out=it_t[:, :nch_b * 8],
                                          in_=sm_p[:, s0:s0 + nch_b * 8])
                        g_t = gp.tile([128, GB_CHUNKS, D], F32, tag="gmain")
                        # split the block into <=GCALL-chunk gather calls
                        for ca in range(0, nch_b, GCALL):
                            cb = min(ca + GCALL, nch_b)
                            nidx = (cb - ca) * 128
                            nc.gpsimd.dma_gather(
                                g_t[:, ca:cb, :],
                                xh_q[p][:, :],
                                it_t[:, ca * 8:cb * 8],
                                nidx,
                                nidx,
                                D,
                                elem_step=D,
                                single_packet=False,
                                queue_num=next_q(),
                            )
                        for mi in range(nch_b // 8):
                            chk = cb0 + mi * 8
                            bi = int(chunk_bucket[chk])
                            if (chunk_bucket[chk:chk + 8] == bi).all():
                                spans = [(0, 8, bi)]
                            else:
                                spans = []
                                j0 = 0
                                for j in range(1, 8):
                                    if chunk_bucket[chk + j] != chunk_bucket[chk + j0]:
                                        spans.append((j0, j, int(chunk_bucket[chk + j0])))
                                        j0 = j
                                spans.append((j0, 8, int(chunk_bucket[chk + j0])))
                            if chk < n_spec_p:
                                # full-width: one mm group = one psum tile
                                ps_t = psp.tile([128, 512], F32, tag="psm")
                                for (ja, jb, bspan) in spans:
                                    nc.tensor.matmul(
                                        out=ps_t[:, ja * 64:jb * 64],
                                        lhsT=SEL[:, int(P.soff[bspan]):int(P.soff[bspan]) + 128],
                                        rhs=g_t[:, mi * 8 + ja:mi * 8 + jb, :],
                                        start=True, stop=True,
                                    )
                                tile_complete = True
                            else:
                                if mm_in_tile == 0:
                                    ps_t = psp.tile([128, 512], F32, tag="psm")
                                slab = mm_in_tile
                                for (ja, jb, bspan) in spans:
                                    nc.tensor.matmul(
                                        out=ps_t[32 * slab:32 * slab + 32, ja * 64:jb * 64],
                                        lhsT=SEL[:, int(P.soff[bspan]):int(P.soff[bspan]) + 32],
                                        rhs=g_t[:, mi * 8 + ja:mi * 8 + jb, :],
                                        start=True, stop=True,
                                        tile_position=(0, 32 * slab),
                                    )
                                mm_in_tile += 1
                                tile_complete = mm_in_tile == 4
                                if tile_complete:
                                    mm_in_tile = 0
                            if tile_complete:
                                if stage_tiles == 0:
                                    stage_t = sp.tile([128, 4 * 512], F32, tag="stg")
                                    stage_row0 = tiles_done * 1024
                                nc.vector.tensor_copy(
                                    out=stage_t[:, stage_tiles * 512:(stage_tiles + 1) * 512],
                                    in_=ps_t[:],
                                )
                                stage_tiles += 1
                                tiles_done += 1
                                flush = (stage_tiles == 4) or (chk + 8 == chunks_p)
                                if flush:
                                    # row(k, q, cc) = row0 + k*1024 + q*8 + cc
                                    nc.sync.dma_start(
                                        out=rp[p][stage_row0:stage_row0 + stage_tiles * 1024, :]
                                        .rearrange("(k q cc) f -> q k cc f", q=128, cc=8),
                                        in_=stage_t[:, :stage_tiles * 512]
                                        .rearrange("q (k cc f) -> q k cc f", cc=8, f=D),
                                    )
                                    stage_tiles = 0
                    assert mm_in_tile == 0, "pass chunks not multiple of 32"

                # ---- recombine: 4 gathers + identity matmul per tile ----
                for t in range(n_rec_tiles):
                    ps2 = psrp.tile([128, 512], F32, tag="psr")
                    for p in range(4):
                        it2 = ip.tile([128, 64], I16, tag="idxr")
                        s0 = (p * NT + t * 1024) // 16
                        nc.sync.dma_start(out=it2[:], in_=sr_p[:, s0:s0 + 64])
                        g2 = grp.tile([128, 8, D], F32, tag="grec")
                        nc.gpsimd.dma_gather(
                            g2[:], rp[p][:, :], it2[:], 1024, 1024, D,
                            elem_step=D, single_packet=False,
                            queue_num=next_q(),
                        )
                        nc.tensor.matmul(
                            out=ps2[:],
                            lhsT=SEL[:, SELW - 128:SELW],
                            rhs=g2[:],
                            start=(p == 0), stop=(p == 3),
                        )
                    nc.vector.tensor_copy(out=B2[:, t * 512:(t + 1) * 512], in_=ps2[:])

                # ---- proximal (node-local) ----
                b0_3 = B0[:].rearrange("p (c f) -> p c f", f=D)
                b1_3 = B1[:].rearrange("p (c f) -> p c f", f=D)
                b2_3 = B2[:].rearrange("p (c f) -> p c f", f=D)
                dv3 = bcast(DINV, CH)
                nc.vector.tensor_tensor(out=b2_3, in0=b2_3, in1=dv3, op=mybir.AluOpType.mult)
                nc.vector.tensor_tensor(out=b1_3, in0=b2_3, in1=b0_3, op=mybir.AluOpType.subtract)
                nc.vector.tensor_tensor(out=b2_3, in0=b1_3, in1=b1_3, op=mybir.AluOpType.mult)
                nc.vector.tensor_reduce(
                    out=RN[:], in_=b2_3, axis=mybir.AxisListType.X, op=mybir.AluOpType.add,
                )
                nc.scalar.sqrt(RN[:], RN[:])
                nc.vector.tensor_scalar_add(RC[:], RN[:], 1e-30)
                nc.vector.reciprocal(RC[:], RC[:])
                nc.vector.tensor_scalar_add(SC[:], RN[:], -LAM)
                nc.vector.tensor_scalar_max(SC[:], SC[:], 0.0)
                nc.vector.tensor_tensor(out=SC[:], in0=SC[:], in1=RC[:], op=mybir.AluOpType.mult)
                nc.vector.tensor_tensor(out=b1_3, in0=b1_3, in1=bcast(SC, CH), op=mybir.AluOpType.mult)
                nc.vector.tensor_tensor(out=b2_3, in0=b1_3, in1=b0_3, op=mybir.AluOpType.add)
                cur = B2

            nc.sync.dma_start(out=out_p[:], in_=B2[:])

    return nc


# ======================================================================
# entry point
# ======================================================================

def _build_and_run(x, edge_index, trace=False):
    x = np.ascontiguousarray(np.asarray(x, dtype=np.float32))
    P = preprocess(x, edge_index)
    nc = build_kernel(P)
    nc.finalize()  # Bacc defers register allocation to compile()
    in_maps = []
    for c in range(CORES):
        d = P.per_core[c]
        in_maps.append({
            "x0": d["x0"], "dinv": d["dinv"], "selectors": P.sel,
            "slots_main": d["slots_main"], "slots_rec": d["slots_rec"],
        })
    res = run_bass_kernel_spmd(nc, in_maps, list(range(CORES)), trace=trace)
    outs = []
    for c in range(CORES):
        o = res.results[c]["out"].reshape(128 * P.CH, D)[:P.NSH]
        outs.append(o)
    return np.concatenate(outs, axis=0), res


def kernel(x, edge_index):
    out, _ = _build_and_run(x, edge_index, trace=False)
    return out



# revision 5
# speedup vs baseline: 2.0258x; 1.0847x over previous
"""AdaptiveConv (GNN message passing) on 8 TRN2 NeuronCores.

Math (the reference simplifies because gamma*2*(1-lambda) == 1):
    deg  = histogram(col) + 1 ; dinv = rsqrt(deg)
    xh   = dinv * x
    spmm(x)[i] = dinv[i] * ( sum_{e: row_e=i} xh[col_e] + xh[i] )
    for 3 iters:  y = spmm(x); d = y - x0; rn = ||d||_row
                  s = relu(rn - lam) / rn;  x = x0 + s*d

Distribution: nodes row-sharded across 8 cores.  Each iteration:
  1. every core computes xh for its shard; FOUR AllGathers (one per
     quarter-of-every-shard sub-table, separate dram tensors) so pass-p
     gathers overlap the remaining collectives (the random graph makes
     the halo dense, so full gathers are optimal)
  2. per-edge gather of 256B source rows (gpsimd.dma_gather, int16 idx
     => 4 sub-tables of <=32767 rows = one "pass" each, <=1024 idx per
     call, single_packet=False).  Edge slots are grouped per dst node
     and padded to a bucket length L; the bucket schedule minimizes
     total 128-slot chunks via suffix-greedy capacity planning with
     node "upgrades" (gathers are DESCRIPTOR-bound at ~8ns/row, so
     slot count is the cost).  Segment-sum via TensorE matmul with
     fixed block-diagonal 0/1 selectors (counts 1-3 get full-width
     M=128/64/42 selectors; counts >=4 get 32-row psum slabs stacked
     4 per tile), fp32 exact
  3. per-pass partial results land in per-pass R tables in HBM
     (psum-schedule order); a fixed-L=1 gather + identity-matmul
     accumulation recombines them into a common node order
  4. proximal step is node-local vector math

Host-side preprocessing only touches edge_index (graph structure): slot
tables, bucket schedule, degree-derived dinv.  All x-dependent compute
runs on device in fp32.
"""

import math
import numpy as np

import concourse.bass as bass
import concourse.mybir as mybir
import concourse.tile as tile
from concourse import bacc
from concourse.bass_utils import run_bass_kernel_spmd

F32 = mybir.dt.float32
I16 = mybir.dt.int16

CORES = 8
D = 64
K_ITERS = 3
LAMBDA_AMP = 0.1
LAM = (1.0 / (2.0 * (1.0 - LAMBDA_AMP))) * LAMBDA_AMP  # = 0.0555...
PAD_ROWS = 12  # zero rows appended to each shard in the gathered table

# (L, M): nodes with count<=L get an aligned L-slot group; M groups per
# 128-slot chunk (M*L <= 128, M <= 32 so psum slabs stack at 32-aligned
# partition offsets).
# first NSPEC buckets are "full-width": one matmul = one psum tile of
# [M, 512] with M > 32 (no slab stacking); R rows 128/chunk uniform
LADDER = [
    (1, 128), (2, 64), (3, 42),
    (4, 32), (5, 25), (6, 21), (7, 18), (8, 16), (9, 14), (10, 12),
    (12, 10), (14, 9), (16, 8), (18, 7), (21, 6), (25, 5), (32, 4),
    (42, 3), (64, 2), (128, 1),
]
NSPEC = 3
NL = len(LADDER)
_LVALS = np.array([l for l, _ in LADDER])

GCALL = 16      # chunks per call (2048 idx; small enough to not hold the
                # engine at ring-drain rate, so 4-queue round-robin overlaps)
GB_CHUNKS = 64  # chunks per gather buffer (matmul consumption block)


def _bucket_of(counts):
    """Vectorized: count (>=1) -> ladder index."""
    return np.searchsorted(_LVALS, counts)


class Plan:
    """Global (core-independent) schedule + per-core data tensors."""

    def __init__(self, N):
        assert N % CORES == 0
        self.N = N
        self.NSH = N // CORES
        assert self.NSH % 4 == 0
        self.QP = self.NSH // 4       # nodes per shard-quarter
        self.SHQ = self.QP + 3        # + zero pad rows per quarter
        self.SUBT = CORES * self.SHQ  # rows per subtable (one quarter of all)
        assert self.SUBT <= 32767, "subtable exceeds int16 index range"
        self.CH = int(math.ceil(self.NSH / 128 / 8)) * 8  # chunks, mult of 8
        self.NT = 128 * self.CH  # padded positions per core


def preprocess(x, edge_index):
    """All graph-structure preprocessing. Returns a Plan."""
    N = x.shape[0]
    P = Plan(N)
    NSH, CH, NT = P.NSH, P.CH, P.NT
    QP, SHQ = P.QP, P.SHQ
    row = np.asarray(edge_index[0], dtype=np.int64)
    col = np.asarray(edge_index[1], dtype=np.int64)

    deg = np.bincount(col, minlength=N).astype(np.float64) + 1.0
    dinv_all = (1.0 / np.sqrt(deg)).astype(np.float32)

    # ---- per-core (node, pass) neighbor lists -------------------------
    core_data = []
    for c in range(CORES):
        m = (row >= c * NSH) & (row < (c + 1) * NSH)
        dl = row[m] - c * NSH
        src = col[m]
        # self loop
        dl = np.concatenate([dl, np.arange(NSH, dtype=np.int64)])
        src = np.concatenate([src, np.arange(c * NSH, (c + 1) * NSH, dtype=np.int64)])
        lcl = src % NSH
        p_of = lcl // QP  # subtable = quarter-of-every-shard
        loc = (src // NSH) * SHQ + (lcl - p_of * QP)  # subtable-local row
        key = dl * 4 + p_of
        order = np.argsort(key, kind="stable")
        key_s = key[order]
        loc_s = loc[order].astype(np.int64)
        cnt = np.bincount(key_s, minlength=NSH * 4).reshape(NSH, 4)
        starts = np.concatenate([[0], np.cumsum(cnt.reshape(-1))])[:-1].reshape(NSH, 4)
        assert cnt.max() <= 128, f"node degree {cnt.max()} exceeds max bucket"
        core_data.append({"cnt": cnt, "starts": starts, "loc_s": loc_s})

    # ---- global schedule -------------------------------------------
    # Minimize total chunks subject to per-core feasibility: a node may
    # be placed in any bucket with L >= its count ("upgrade"), so the
    # binding constraints are the suffix capacities.  Greedy from the
    # largest bucket down is optimal since M is larger at lower buckets.
    nch = np.zeros((4, NL), dtype=np.int64)
    Ms = np.array([m for _, m in LADDER])
    for p in range(4):
        nat = np.zeros((CORES, NL), dtype=np.int64)
        for c in range(CORES):
            cnts = core_data[c]["cnt"][:, p]
            nz = cnts[cnts > 0]
            nat[c] = np.bincount(_bucket_of(nz), minlength=NL)
        suf_need = np.maximum.reduce(
            [np.cumsum(nat[c][::-1])[::-1] for c in range(CORES)])
        cap = 0
        for t in range(NL - 1, -1, -1):
            deficit = max(0, int(suf_need[t]) - cap)
            nch[p][t] = -(-deficit // int(Ms[t]))
            cap += int(Ms[t] * nch[p][t])
    # pad special section to mult of 8 chunks, standard to mult of 32
    n_spec = np.zeros(4, dtype=np.int64)
    for p in range(4):
        cs = int(nch[p][:NSPEC].sum())
        pad = (-cs) % 8
        nch[p][NSPEC - 1] += pad
        n_spec[p] = cs + pad
        cstd = int(nch[p][NSPEC:].sum())
        pad = (-cstd) % 32
        nch[p][NSPEC] += pad
    ch_tot = nch.sum(axis=1)
    P.nch = nch
    P.ch_tot = ch_tot
    P.n_spec = n_spec
    # R rows: 1024 per staged psum tile (special: 8 chunks x 128 rows;
    # standard: 32 chunks x 32 rows)
    P.rrows = [int(1024 * (n_spec[p] // 8 + (ch_tot[p] - n_spec[p]) // 32))
               for p in range(4)]
    for p in range(4):
        assert P.rrows[p] + 1 <= 32767, f"R table {p} too big: {P.rrows[p]}"

    # ---- per-core slot arrays + recombine tables ----------------------
    per_core = []
    for c in range(CORES):
        cd = core_data[c]
        cnt, starts, loc_s = cd["cnt"], cd["starts"], cd["loc_s"]
        slots_main = []
        kp = np.full((4, NSH), -1, dtype=np.int64)  # R_p row of node l
        for p in range(4):
            slots_p = np.full(int(ch_tot[p]) * 128, QP, dtype=np.int16)  # ZROW=QP
            cnts = cnt[:, p]
            nodes_nz = np.nonzero(cnts > 0)[0]
            # capacity-fill: largest buckets take the largest counts;
            # smaller nodes may be upgraded into leftover capacity
            order = nodes_nz[np.argsort(-cnts[nodes_nz], kind="stable")]
            chunk_base_of = np.concatenate([[0], np.cumsum(nch[p])])
            ptr = 0
            for bi in range(NL - 1, -1, -1):
                L, M = LADDER[bi]
                cap_b = int(nch[p][bi]) * M
                take = order[ptr:ptr + cap_b]
                ptr += cap_b
                if len(take) == 0:
                    continue
                assert cnts[take[0]] <= L, "schedule infeasible"
                cb = int(chunk_base_of[bi])
                nsp = int(P.n_spec[p])
                for j, l in enumerate(take):
                    chk = cb + j // M
                    g = j % M
                    n = int(cnts[l])
                    s0 = chk * 128 + g * L
                    slots_p[s0:s0 + n] = loc_s[starts[l, p]:starts[l, p] + n]
                    if bi < NSPEC:
                        # full-width: tile = chk//8, row = g*8 + cc
                        kp[p, l] = (chk // 8) * 1024 + g * 8 + (chk % 8)
                    else:
                        # 4-slab: relative to the standard section
                        cs = chk - nsp
                        mm = cs // 8
                        kp[p, l] = (nsp // 8) * 1024 + (mm // 4) * 1024                             + ((mm % 4) * 32 + g) * 8 + (cs % 8)
            assert ptr >= len(order), "nodes left unplaced"
            slots_main.append(slots_p)
        slots_main = np.concatenate(slots_main)

        # recombine: slot (ch, e) -> position r = e*CH + ch -> node l=r
        e_idx = np.arange(NT, dtype=np.int64)
        ch_i = e_idx // 128
        e_i = e_idx % 128
        pos = e_i * CH + ch_i
        slots_rec = []
        for p in range(4):
            zr = P.rrows[p]
            v = np.full(NT, zr, dtype=np.int64)
            real = pos < NSH
            l_of = pos[real]
            kv = kp[p, l_of]
            v[real] = np.where(kv >= 0, kv, zr)
            slots_rec.append(v.astype(np.int16))
        slots_rec = np.concatenate(slots_rec)

        # x0 / dinv tiles in position layout [128, CH*D] / [128, CH]
        xt = np.zeros((128 * CH, D), dtype=np.float32)
        xt[:NSH] = x[c * NSH:(c + 1) * NSH]
        dt_ = np.zeros(128 * CH, dtype=np.float32)
        dt_[:NSH] = dinv_all[c * NSH:(c + 1) * NSH]
        per_core.append({
            "x0": np.ascontiguousarray(xt.reshape(128, CH * D)),
            "dinv": np.ascontiguousarray(dt_.reshape(128, CH)),
            "slots_main": _wrap16(slots_main),
            "slots_rec": _wrap16(slots_rec),
        })
    P.per_core = per_core

    # selector blob: full-width buckets get M columns, standard get 32
    widths = [128 if bi < NSPEC else 32 for bi, (L, M) in enumerate(LADDER)]
    soff = np.concatenate([[0], np.cumsum(widths)]).astype(int)
    sel = np.zeros((128, int(soff[-1]) + 128), dtype=np.float32)
    e = np.arange(128)
    for bi, (L, M) in enumerate(LADDER):
        g = e // L
        ok = g < M
        sel[e[ok], soff[bi] + g[ok]] = 1.0
    sel[e, int(soff[-1]) + e] = 1.0  # identity for recombine
    P.sel = sel
    P.soff = soff
    return P


def _wrap16(a):
    """int16 1-D array -> [128, ceil(n/16)] wrapped layout: value at
    (p, s) = a[s*16 + p%16], replicated across the 8 Q7 core stripes."""
    n = len(a)
    n16 = int(math.ceil(n / 16)) * 16
    b = np.zeros(n16, dtype=np.int16)
    b[:n] = a
    w = b.reshape(-1, 16).T
    return np.ascontiguousarray(np.tile(w, (8, 1)))


# ======================================================================
# Bass kernel builder
# ======================================================================

def build_kernel(P: Plan):
    NSH, SUBT, CH, NT = P.NSH, P.SUBT, P.CH, P.NT
    QP, SHQ = P.QP, P.SHQ
    CHD = CH * D
    TOTM = P.per_core[0]["slots_main"].shape[1]
    TOTR = P.per_core[0]["slots_rec"].shape[1]
    n_rec_tiles = CH // 8

    nc = bacc.Bacc(None, target_bir_lowering=False, num_swdge_queues=4)
    qrr = [0]  # round-robin SWDGE queue counter

    def next_q():
        q = qrr[0] & 3
        qrr[0] += 1
        return q

    x0_p = nc.declare_dram_parameter("x0", [128, CHD], F32, isOutput=False)
    dinv_p = nc.declare_dram_parameter("dinv", [128, CH], F32, isOutput=False)
    SELW = P.sel.shape[1]
    sel_p = nc.declare_dram_parameter("selectors", [128, SELW], F32, isOutput=False)
    sm_p = nc.declare_dram_parameter("slots_main", [128, TOTM], I16, isOutput=False)
    sr_p = nc.declare_dram_parameter("slots_rec", [128, TOTR], I16, isOutput=False)
    out_p = nc.declare_dram_parameter("out", [128, CHD], F32, isOutput=True)

    bounce_q = [nc.dram_tensor(f"bounce{p}", [SHQ, D], F32) for p in range(4)]
    xh_q = [nc.dram_tensor(f"xhq{p}", [SUBT, D], F32, addr_space="Shared")
            for p in range(4)]
    rp = [nc.dram_tensor(f"rp{p}", [P.rrows[p] + 1, D], F32) for p in range(4)]

    with tile.TileContext(nc) as tc:
        with (
            tc.tile_pool(name="persist", bufs=1) as pp,
            tc.tile_pool(name="gmain", bufs=2) as gp,
            tc.tile_pool(name="grec", bufs=8) as grp,
            tc.tile_pool(name="idx", bufs=2) as ip,
            tc.tile_pool(name="stage", bufs=2) as sp,
            tc.tile_pool(name="psum", bufs=2, space="PSUM") as psp,
            tc.tile_pool(name="psumr", bufs=2, space="PSUM") as psrp,
        ):
            B0 = pp.tile([128, CHD], F32)
            B1 = pp.tile([128, CHD], F32)
            B2 = pp.tile([128, CHD], F32)
            DINV = pp.tile([128, CH], F32)
            SEL = pp.tile([128, SELW], F32)
            RN = pp.tile([128, CH], F32)
            SC = pp.tile([128, CH], F32)
            RC = pp.tile([128, CH], F32)
            ZT = pp.tile([1, max(PAD_ROWS * D, D)], F32)

            nc.sync.dma_start(out=B0[:], in_=x0_p[:])
            nc.sync.dma_start(out=DINV[:], in_=dinv_p[:])
            nc.sync.dma_start(out=SEL[:], in_=sel_p[:])
            nc.vector.memset(ZT[:], 0.0)
            for p in range(4):
                nc.sync.dma_start(
                    out=bounce_q[p][QP:SHQ, :].rearrange("(o r) f -> o (r f)", o=1),
                    in_=ZT[:1, :3 * D])
                nc.sync.dma_start(
                    out=rp[p][P.rrows[p]:P.rrows[p] + 1, :], in_=ZT[:1, :D])

            def bcast(t, cols):
                """[128, cols] tile -> [128, cols, D] zero-stride AP."""
                return t[:].rearrange("p (c o) -> p c o", o=1).to_broadcast([128, cols, D])

            def bounce_pieces(p):
                """affine DMA pieces covering positions [p*QP, (p+1)*QP)."""
                pieces = []
                a, b = p * QP, (p + 1) * QP
                base = 0
                # partial head
                if a % CH:
                    g = a // CH
                    take = min(CH - a % CH, b - a)
                    pieces.append((base, g, g + 1, a % CH, a % CH + take))
                    base += take
                    a += take
                # full middle
                gm0, gm1 = a // CH, b // CH
                if gm1 > gm0:
                    pieces.append((base, gm0, gm1, 0, CH))
                    base += (gm1 - gm0) * CH
                    a = gm1 * CH
                if a < b:  # partial tail
                    pieces.append((base, b // CH, b // CH + 1, 0, b - a))
                return pieces

            cur = B0
            for it in range(K_ITERS):
                # ---- xh = dinv * x_cur -> B1 -> bounce -> AllGather ----
                nc.vector.tensor_tensor(
                    out=B1[:].rearrange("p (c f) -> p c f", f=D),
                    in0=cur[:].rearrange("p (c f) -> p c f", f=D),
                    in1=bcast(DINV, CH),
                    op=mybir.AluOpType.mult,
                )
                for p in range(4):
                    for (rbase, g0, g1, c0, c1) in bounce_pieces(p):
                        n = (g1 - g0) * (c1 - c0)
                        nc.sync.dma_start(
                            out=bounce_q[p][rbase:rbase + n, :]
                            .rearrange("(g c) f -> g c f", g=g1 - g0),
                            in_=B1[g0:g1, c0 * D:c1 * D]
                            .rearrange("g (c f) -> g c f", f=D),
                        )
                    nc.gpsimd.collective_compute(
                        "AllGather",
                        mybir.AluOpType.bypass,
                        replica_groups=[list(range(CORES))],
                        ins=[bounce_q[p][:, :]],
                        outs=[xh_q[p][:, :]],
                    )

                # ---- main passes: gather + selector matmuls -> R_p ----
                for p in range(4):
                    pass_chunk0 = int(np.sum(P.ch_tot[:p]))
                    chunks_p = int(P.ch_tot[p])
                    chunk_bucket = np.repeat(np.arange(NL), P.nch[p])
                    assert len(chunk_bucket) == chunks_p
                    n_blocks = int(math.ceil(chunks_p / GB_CHUNKS))
                    n_spec_p = int(P.n_spec[p])
                    mm_in_tile = 0
                    ps_t = None
                    stage_t = None
                    stage_tiles = 0
                    stage_row0 = 0
                    tiles_done = 0
                    for blk in range(n_blocks):
                        cb0 = blk * GB_CHUNKS
                        nch_b = min(GB_CHUNKS, chunks_p - cb0)
                        it_t = ip.tile([128, GB_CHUNKS * 8], I16, tag="idxm")
                        s0 = (pass_chunk0 + cb0) * 8
                        nc.sync.dma_start(out=it_t[:, :nch_b * 8],
                                          in_=sm_p[:, s0:s0 + nch_b * 8])
                        g_t = gp.tile([128, GB_CHUNKS, D], F32, tag="gmain")
                        # split the block into <=GCALL-chunk gather calls
                        for ca in range(0, nch_b, GCALL):
                            cb = min(ca + GCALL, nch_b)
                            nidx = (cb - ca) * 128
                            nc.gpsimd.dma_gather(
                                g_t[:, ca:cb, :],
                                xh_q[p][:, :],
                                it_t[:, ca * 8:cb * 8],
                                nidx,
                                nidx,
                                D,
                                elem_step=D,
                                single_packet=False,
                                queue_num=next_q(),
                            )
                        for mi in range(nch_b // 8):
                            chk = cb0 + mi * 8
                            bi = int(chunk_bucket[chk])
                            if (chunk_bucket[chk:chk + 8] == bi).all():
                                spans = [(0, 8, bi)]
                            else:
                                spans = []
                                j0 = 0
                                for j in range(1, 8):
                                    if chunk_bucket[chk + j] != chunk_bucket[chk + j0]:
                                        spans.append((j0, j, int(chunk_bucket[chk + j0])))
                                        j0 = j
                                spans.append((j0, 8, int(chunk_bucket[chk + j0])))
                            if chk < n_spec_p:
                                # full-width: one mm group = one psum tile
                                ps_t = psp.tile([128, 512], F32, tag="psm")
                                for (ja, jb, bspan) in spans:
                                    nc.tensor.matmul(
                                        out=ps_t[:, ja * 64:jb * 64],
                                        lhsT=SEL[:, int(P.soff[bspan]):int(P.soff[bspan]) + 128],
                                        rhs=g_t[:, mi * 8 + ja:mi * 8 + jb, :],
                                        start=True, stop=True,
                                    )
                                tile_complete = True
                            else:
                                if mm_in_tile == 0:
                                    ps_t = psp.tile([128, 512], F32, tag="psm")
                                slab = mm_in_tile
                                for (ja, jb, bspan) in spans:
                                    nc.tensor.matmul(
                                        out=ps_t[32 * slab:32 * slab + 32, ja * 64:jb * 64],
                                        lhsT=SEL[:, int(P.soff[bspan]):int(P.soff[bspan]) + 32],
                                        rhs=g_t[:, mi * 8 + ja:mi * 8 + jb, :],
                                        start=True, stop=True,
                                        tile_position=(0, 32 * slab),
                                    )
                                mm_in_tile += 1
                                tile_complete = mm_in_tile == 4
                                if tile_complete:
                                    mm_in_tile = 0
                            if tile_complete:
                                if stage_tiles == 0:
                                    stage_t = sp.tile([128, 4 * 512], F32, tag="stg")
                                    stage_row0 = tiles_done * 1024
                                nc.vector.tensor_copy(
                                    out=stage_t[:, stage_tiles * 512:(stage_tiles + 1) * 512],
                                    in_=ps_t[:],
                                )
                                stage_tiles += 1
                                tiles_done += 1
                                flush = (stage_tiles == 4) or (chk + 8 == chunks_p)
                                if flush:
                                    # row(k, q, cc) = row0 + k*1024 + q*8 + cc
                                    nc.sync.dma_start(
                                        out=rp[p][stage_row0:stage_row0 + stage_tiles * 1024, :]
                                        .rearrange("(k q cc) f -> q k cc f", q=128, cc=8),
                                        in_=stage_t[:, :stage_tiles * 512]
                                        .rearrange("q (k cc f) -> q k cc f", cc=8, f=D),
                                    )
                                    stage_tiles = 0
                    assert mm_in_tile == 0, "pass chunks not multiple of 32"

                # ---- recombine: 4 gathers + identity matmul per tile ----
                for t in range(n_rec_tiles):
                    ps2 = psrp.tile([128, 512], F32, tag="psr")
                    for p in range(4):
                        it2 = ip.tile([128, 64], I16, tag="idxr")
                        s0 = (p * NT + t * 1024) // 16
                        nc.sync.dma_start(out=it2[:], in_=sr_p[:, s0:s0 + 64])
                        g2 = grp.tile([128, 8, D], F32, tag="grec")
                        nc.gpsimd.dma_gather(
                            g2[:], rp[p][:, :], it2[:], 1024, 1024, D,
                            elem_step=D, single_packet=False,
                            queue_num=next_q(),
                        )
                        nc.tensor.matmul(
                            out=ps2[:],
                            lhsT=SEL[:, SELW - 128:SELW],
                            rhs=g2[:],
                            start=(p == 0), stop=(p == 3),
                        )
                    nc.vector.tensor_copy(out=B2[:, t * 512:(t + 1) * 512], in_=ps2[:])

                # ---- proximal (node-local) ----
                b0_3 = B0[:].rearrange("p (c f) -> p c f", f=D)
                b1_3 = B1[:].rearrange("p (c f) -> p c f", f=D)
                b2_3 = B2[:].rearrange("p (c f) -> p c f", f=D)
                dv3 = bcast(DINV, CH)
                nc.vector.tensor_tensor(out=b2_3, in0=b2_3, in1=dv3, op=mybir.AluOpType.mult)
                nc.vector.tensor_tensor(out=b1_3, in0=b2_3, in1=b0_3, op=mybir.AluOpType.subtract)
                nc.vector.tensor_tensor(out=b2_3, in0=b1_3, in1=b1_3, op=mybir.AluOpType.mult)
                nc.vector.tensor_reduce(
                    out=RN[:], in_=b2_3, axis=mybir.AxisListType.X, op=mybir.AluOpType.add,
                )
                nc.scalar.sqrt(RN[:], RN[:])
                nc.vector.tensor_scalar_add(RC[:], RN[:], 1e-30)
                nc.vector.reciprocal(RC[:], RC[:])
                nc.vector.tensor_scalar_add(SC[:], RN[:], -LAM)
                nc.vector.tensor_scalar_max(SC[:], SC[:], 0.0)
                nc.vector.tensor_tensor(out=SC[:], in0=SC[:], in1=RC[:], op=mybir.AluOpType.mult)
                nc.vector.tensor_tensor(out=b1_3, in0=b1_3, in1=bcast(SC, CH), op=mybir.AluOpType.mult)
                nc.vector.tensor_tensor(out=b2_3, in0=b1_3, in1=b0_3, op=mybir.AluOpType.add)
                cur = B2

            nc.sync.dma_start(out=out_p[:], in_=B2[:])

    return nc


# ======================================================================
# entry point
# ======================================================================

def _build_and_run(x, edge_index, trace=False):
    x = np.ascontiguousarray(np.asarray(x, dtype=np.float32))
    P = preprocess(x, edge_index)
    nc = build_kernel(P)
    nc.finalize()  # Bacc defers register allocation to compile()
    in_maps = []
    for c in range(CORES):
        d = P.per_core[c]
        in_maps.append({
            "x0": d["x0"], "dinv": d["dinv"], "selectors": P.sel,
            "slots_main": d["slots_main"], "slots_rec": d["slots_rec"],
        })
    res = run_bass_kernel_spmd(nc, in_maps, list(range(CORES)), trace=trace)
    outs = []
    for c in range(CORES):
        o = res.results[c]["out"].reshape(128 * P.CH, D)[:P.NSH]
        outs.append(o)
    return np.concatenate(outs, axis=0), res


def kernel(x, edge_index):
    out, _ = _build_and_run(x, edge_index, trace=False)
    return out

